# revision 4
# baseline (speedup 1.0000x reference)
"""Trainium2 Bass kernel for nn_ComboLoss (MTP loss + BCE loss).

Data-parallel over 8 NeuronCores: each core processes 8192 rows of the
65536-row batch and produces two partial sums [sum(ce + reg), sum(bce_raw)];
the host combines them into the final scalar loss.

Per-core layout: 8 "supertiles" of 1024 rows; each supertile maps G=8
consecutive rows onto each of the 128 SBUF partitions (so row r of the
supertile lives at partition r//8, group r%8).  All per-row math is done
with row-batch on the partition axis and (group, mode, waypoint) unrolled
along the free axis, so every instruction covers 1024 rows.
"""

import math
import os
import sys
from contextlib import ExitStack

import numpy as np

for _p in ("/opt/trn_rl_repo", "/root/.axon_site/_ro/trn_rl_repo"):
    if os.path.isdir(_p) and _p not in sys.path:
        sys.path.insert(0, _p)
        break

import concourse.bass as bass
import concourse.bacc as bacc
import concourse.mybir as mybir
import concourse.tile as tile
from concourse.bass_utils import run_bass_kernel_spmd

F32 = mybir.dt.float32
I32 = mybir.dt.int32
ALU = mybir.AluOpType
ACTF = mybir.ActivationFunctionType
AX = mybir.AxisListType

B = 65536
NCORES = 8
BLOC = B // NCORES          # 8192 rows per core
P = 128                     # SBUF partitions
G = 8                       # row-groups per partition per supertile
ROWS_SUP = P * G            # 1024 rows per supertile
NSUP = BLOC // ROWS_SUP     # 8 supertiles
NM = 5                      # modes
T = 50                      # waypoints
T2 = 2 * T                  # 100 coords per trajectory
F = NM * T2 + NM            # 505 features in path_pred
NJ = NSUP * G               # 64 row-groups per partition over the whole core

BIG = 1.0e30
INV_COS5SQ = float(1.0 / (math.cos(math.radians(5.0)) ** 2))
T2PAD = T2 + 4              # padded stride for the gathered-trajectory tile


def _build_bass():
    nc = bacc.Bacc("TRN2", target_bir_lowering=False, debug=False)

    pred_d = nc.dram_tensor("path_pred", [BLOC, F], F32, kind="ExternalInput").ap()
    gt_d = nc.dram_tensor("path_gt", [BLOC, T2], F32, kind="ExternalInput").ap()
    crp_d = nc.dram_tensor("cr_pred", [P, NJ], F32, kind="ExternalInput").ap()
    crg_d = nc.dram_tensor("cr_gt", [P, NJ], F32, kind="ExternalInput").ap()
    rnd_d = nc.dram_tensor("rand_modes", [P, NJ], F32, kind="ExternalInput").ap()
    out_d = nc.dram_tensor("partials", [1, 2], F32, kind="ExternalOutput").ap()

    with tile.TileContext(nc) as tc, ExitStack() as ctx:
        cpool = ctx.enter_context(tc.tile_pool(name="const", bufs=1))
        inp = ctx.enter_context(tc.tile_pool(name="inp", bufs=3))
        wrk = ctx.enter_context(tc.tile_pool(name="wrk", bufs=2))
        sml = ctx.enter_context(tc.tile_pool(name="sml", bufs=2))
        pps = ctx.enter_context(tc.tile_pool(name="pps", bufs=1, space="PSUM"))

        # ---- constants / residents ----
        iota_i = cpool.tile([P, NM], I32)
        nc.gpsimd.iota(iota_i[:], pattern=[[1, NM]], base=0, channel_multiplier=0)
        iota_a = cpool.tile([P, NM], F32)          # [0,1,2,3,4]
        nc.vector.tensor_copy(iota_a[:], iota_i[:])
        iota_di = cpool.tile([P, NM], I32)
        nc.gpsimd.iota(iota_di[:], pattern=[[-1, NM]], base=NM, channel_multiplier=0)
        iota_d = cpool.tile([P, NM], F32)          # [5,4,3,2,1]
        nc.vector.tensor_copy(iota_d[:], iota_di[:])
        ones = cpool.tile([P, 1], F32)
        nc.vector.memset(ones[:], 1.0)
        negone = cpool.tile([P, 1], F32)
        nc.vector.memset(negone[:], -1.0)

        rnd_sb = cpool.tile([P, NJ], F32)
        nc.sync.dma_start(rnd_sb[:], rnd_d)
        crp_sb = cpool.tile([P, NJ], F32)
        nc.sync.dma_start(crp_sb[:], crp_d)
        crg_sb = cpool.tile([P, NJ], F32)
        nc.sync.dma_start(crg_sb[:], crg_d)

        regB = cpool.tile([P, NJ], F32)            # per-row smooth-L1 reg
        mbB = cpool.tile([P, NJ], F32)             # per-row (max_logit - best_logit)
        shB = cpool.tile([P, NJ * NM], F32)        # shifted logits for exp pass
        stack2 = cpool.tile([P, 2], F32)

        iotaA_b = iota_a[:].unsqueeze(1).broadcast_to((P, G, NM))
        iotaD_b = iota_d[:].unsqueeze(1).broadcast_to((P, G, NM))

        # ================= Phase A: per-supertile main loss =================
        for i in range(NSUP):
            rsl = slice(i * ROWS_SUP, (i + 1) * ROWS_SUP)

            pred_t = inp.tile([P, G * F], F32, tag="pred")
            nc.sync.dma_start(
                pred_t[:], pred_d[rsl, :].rearrange("(p g) f -> p (g f)", p=P)
            )
            gt_t = inp.tile([P, G * T2], F32, tag="gt")
            nc.sync.dma_start(
                gt_t[:], gt_d[rsl, :].rearrange("(p g) f -> p (g f)", p=P)
            )

            predg = pred_t[:].rearrange("p (g f) -> p g f", g=G)
            traj4 = predg[:, :, 0:NM * T2].rearrange("p g (m t) -> p g m t", m=NM)
            logits = predg[:, :, NM * T2:F]                     # (P, G, NM)
            gt3 = gt_t[:].rearrange("p (g t) -> p g t", g=G)    # (P, G, T2)

            # --- trajectory deltas (all modes) ---
            d_t = wrk.tile([P, G * NM * T2], F32, tag="d")
            d4 = d_t[:].rearrange("p (g m t) -> p g m t", g=G, m=NM)
            for m in range(NM):
                nc.vector.tensor_sub(d4[:, :, m, :], traj4[:, :, m, :], gt3)

            # --- squared deltas, per-waypoint squared distance, sqrt, sum_t ---
            s_t = wrk.tile([P, G * NM * T2], F32, tag="s")
            nc.scalar.activation(s_t[:], d_t[:], ACTF.Square)
            s4 = s_t[:].rearrange("p (gm t c) -> p gm t c", gm=G * NM, t=T, c=2)
            e_t = wrk.tile([P, G * NM * T], F32, tag="e")
            e3 = e_t[:].rearrange("p (gm t) -> p gm t", gm=G * NM)
            nc.vector.tensor_add(e3, s4[:, :, :, 0], s4[:, :, :, 1])
            nc.scalar.activation(e_t[:], e_t[:], ACTF.Sqrt)     # in-place sqrt
            dist = sml.tile([P, G * NM], F32, tag="dist")
            nc.vector.tensor_reduce(dist[:], e3, axis=AX.X, op=ALU.add)
            dist3 = dist[:].rearrange("p (g m) -> p g m", g=G)

            # --- eligibility: angle(ref_last, traj_last) <= 5deg ---
            tl2 = traj4[:, :, :, T2 - 2:T2]                     # (P,G,NM,2)
            sql = sml.tile([P, G * NM * 2], F32, tag="sql")
            sql4 = sql[:].rearrange("p (g m c) -> p g m c", g=G, m=NM)
            nc.scalar.activation(sql4, tl2, ACTF.Square)
            nt2 = sml.tile([P, G * NM], F32, tag="nt2")
            nt23 = nt2[:].rearrange("p (g m) -> p g m", g=G)
            nc.vector.tensor_add(nt23, sql4[:, :, :, 0], sql4[:, :, :, 1])

            gl2 = gt3[:, :, T2 - 2:T2]                          # (P,G,2)
            gg = sml.tile([P, G * 2], F32, tag="gg")
            gg3 = gg[:].rearrange("p (g c) -> p g c", g=G)
            nc.vector.tensor_mul(gg3, gl2, gl2)
            nr2 = sml.tile([P, G], F32, tag="nr2")
            nc.vector.tensor_add(nr2[:], gg3[:, :, 0], gg3[:, :, 1])

            tx = traj4[:, :, :, T2 - 2]                         # (P,G,NM)
            ty = traj4[:, :, :, T2 - 1]
            rx_b = gt3[:, :, T2 - 2:T2 - 1].broadcast_to((P, G, NM))
            ry_b = gt3[:, :, T2 - 1:T2].broadcast_to((P, G, NM))
            a1 = sml.tile([P, G * NM], F32, tag="a1")
            a13 = a1[:].rearrange("p (g m) -> p g m", g=G)
            nc.vector.tensor_mul(a13, tx, rx_b)
            a2 = sml.tile([P, G * NM], F32, tag="a2")
            a23 = a2[:].rearrange("p (g m) -> p g m", g=G)
            nc.vector.tensor_mul(a23, ty, ry_b)
            dot = sml.tile([P, G * NM], F32, tag="dot")
            dot3 = dot[:].rearrange("p (g m) -> p g m", g=G)
            nc.vector.tensor_add(dot3, a13, a23)

            rhs = sml.tile([P, G * NM], F32, tag="rhs")
            rhs3 = rhs[:].rearrange("p (g m) -> p g m", g=G)
            nr2_b = nr2[:].unsqueeze(2).broadcast_to((P, G, NM))
            nc.vector.tensor_mul(rhs3, nt23, nr2_b)
            dot2c = sml.tile([P, G * NM], F32, tag="dot2c")
            nc.vector.scalar_tensor_tensor(
                dot2c[:], dot[:], INV_COS5SQ, dot[:], ALU.mult, ALU.mult
            )
            e1 = sml.tile([P, G * NM], F32, tag="e1")
            nc.vector.tensor_tensor(e1[:], dot2c[:], rhs[:], ALU.is_ge)
            elig = sml.tile([P, G * NM], F32, tag="elig")
            nc.vector.scalar_tensor_tensor(
                elig[:], dot[:], 0.0, e1[:], ALU.is_gt, ALU.mult
            )

            # --- score = dist where eligible else BIG; argmin over modes ---
            welig = sml.tile([P, G * NM], F32, tag="welig")
            nc.vector.tensor_scalar(welig[:], elig[:], -BIG, BIG, ALU.mult, ALU.add)
            score = sml.tile([P, G * NM], F32, tag="score")
            score3 = score[:].rearrange("p (g m) -> p g m", g=G)
            nc.vector.tensor_add(score[:], dist[:], welig[:])
            minv = sml.tile([P, G], F32, tag="minv")
            nc.vector.tensor_reduce(minv[:], score3, axis=AX.X, op=ALU.min)
            eq = sml.tile([P, G * NM], F32, tag="eq")
            eq3 = eq[:].rearrange("p (g m) -> p g m", g=G)
            minv_b = minv[:].unsqueeze(2).broadcast_to((P, G, NM))
            nc.vector.tensor_tensor(eq3, score3, minv_b, ALU.is_equal)
            wq = sml.tile([P, G * NM], F32, tag="wq")
            wq3 = wq[:].rearrange("p (g m) -> p g m", g=G)
            nc.vector.tensor_tensor(wq3, eq3, iotaD_b, ALU.mult)
            mxw = sml.tile([P, G], F32, tag="mxw")
            nc.vector.tensor_reduce(mxw[:], wq3, axis=AX.X, op=ALU.max)
            bidx = sml.tile([P, G], F32, tag="bidx")
            nc.vector.tensor_scalar(bidx[:], mxw[:], -1.0, float(NM), ALU.mult, ALU.add)
            anye = sml.tile([P, G], I32, tag="anye")
            nc.vector.tensor_scalar(anye[:], minv[:], BIG, None, ALU.is_lt)
            bf = sml.tile([P, G], F32, tag="bf")
            nc.vector.tensor_copy(bf[:], rnd_sb[:, i * G:(i + 1) * G])
            nc.vector.copy_predicated(bf[:], anye[:], bidx[:])

            # --- one-hot mask of best mode; gather best-mode deltas ---
            mask = sml.tile([P, G * NM], I32, tag="mask")
            mask3 = mask[:].rearrange("p (g m) -> p g m", g=G)
            bf_b = bf[:].unsqueeze(2).broadcast_to((P, G, NM))
            nc.vector.tensor_tensor(mask3, iotaA_b, bf_b, ALU.is_equal)

            bd = wrk.tile([P, G * T2PAD], F32, tag="bd")
            bd3 = bd[:].rearrange("p (g t) -> p g t", g=G)[:, :, 0:T2]
            nc.vector.tensor_copy(bd3, d4[:, :, 0, :])
            for m in range(1, NM):
                mk = mask3[:, :, m:m + 1].broadcast_to((P, G, T2))
                nc.vector.copy_predicated(bd3, mk, d4[:, :, m, :])

            # --- smooth-L1 of gathered deltas:
            #     sum(huber) = sum(relu(|d|-1)) + 0.5*sum(min(|d|,1)^2) ---
            ad_t = wrk.tile([P, G * T2], F32, tag="ad")
            ad3 = ad_t[:].rearrange("p (g t) -> p g t", g=G)
            nc.scalar.activation(ad3, bd3, ACTF.Abs)
            t_t = wrk.tile([P, G * T2], F32, tag="t")
            nc.scalar.activation(t_t[:], ad_t[:], ACTF.Relu, bias=negone[:])
            tred = sml.tile([P, G], F32, tag="tred")
            nc.vector.tensor_reduce(
                tred[:], t_t[:].rearrange("p (g t) -> p g t", g=G), axis=AX.X, op=ALU.add
            )
            q_t = wrk.tile([P, G * T2], F32, tag="q")
            nc.vector.tensor_scalar(q_t[:], ad_t[:], 1.0, None, ALU.min)
            nc.scalar.activation(t_t[:], q_t[:], ACTF.Square)
            qred = sml.tile([P, G], F32, tag="qred")
            nc.vector.tensor_reduce(
                qred[:], t_t[:].rearrange("p (g t) -> p g t", g=G), axis=AX.X, op=ALU.add
            )
            tsa = sml.tile([P, G], F32, tag="tsa")
            nc.vector.tensor_scalar(tsa[:], tred[:], 1.0 / T2, None, ALU.mult)
            nc.vector.scalar_tensor_tensor(
                regB[:, i * G:(i + 1) * G], qred[:], 0.5 / T2, tsa[:], ALU.mult, ALU.add
            )

            # --- cross-entropy pieces (exp/ln deferred to phase B) ---
            mxl = sml.tile([P, G], F32, tag="mxl")
            nc.vector.tensor_reduce(mxl[:], logits, axis=AX.X, op=ALU.max)
            shs = shB[:, i * G * NM:(i + 1) * G * NM].rearrange(
                "p (g m) -> p g m", g=G
            )
            mxl_b = mxl[:].unsqueeze(2).broadcast_to((P, G, NM))
            nc.vector.tensor_sub(shs, logits, mxl_b)
            lbt = sml.tile([P, G * NM], F32, tag="lbt")
            lbt3 = lbt[:].rearrange("p (g m) -> p g m", g=G)
            nc.vector.tensor_mul(lbt3, logits, mask3)
            lb = sml.tile([P, G], F32, tag="lb")
            nc.vector.tensor_reduce(lb[:], lbt3, axis=AX.X, op=ALU.add)
            nc.vector.tensor_sub(mbB[:, i * G:(i + 1) * G], mxl[:], lb[:])

        # ================= Phase B: exp/ln pass + BCE + final =================
        ex = cpool.tile([P, NJ * NM], F32)
        nc.scalar.activation(ex[:], shB[:], ACTF.Exp)
        se = cpool.tile([P, NJ], F32)
        nc.vector.tensor_reduce(
            se[:], ex[:].rearrange("p (j m) -> p j m", j=NJ), axis=AX.X, op=ALU.add
        )
        nc.scalar.activation(se[:], se[:], ACTF.Ln)             # in-place lse
        ce = cpool.tile([P, NJ], F32)
        nc.vector.tensor_add(ce[:], mbB[:], se[:])
        nc.vector.tensor_add(ce[:], ce[:], regB[:])             # ce + reg
        nc.vector.tensor_reduce(stack2[:, 0:1], ce[:], axis=AX.X, op=ALU.add)

        lp = cpool.tile([P, NJ], F32)
        nc.scalar.activation(lp[:], crp_sb[:], ACTF.Ln)
        nc.vector.tensor_scalar(lp[:], lp[:], -100.0, None, ALU.max)
        om = cpool.tile([P, NJ], F32)
        nc.vector.tensor_scalar(om[:], crp_sb[:], -1.0, 1.0, ALU.mult, ALU.add)
        nc.scalar.activation(om[:], om[:], ACTF.Ln)
        nc.vector.tensor_scalar(om[:], om[:], -100.0, None, ALU.max)
        u_t = cpool.tile([P, NJ], F32)
        nc.vector.tensor_sub(u_t[:], lp[:], om[:])
        nc.vector.tensor_mul(u_t[:], crg_sb[:], u_t[:])
        nc.vector.tensor_add(u_t[:], u_t[:], om[:])
        nc.vector.tensor_reduce(stack2[:, 1:2], u_t[:], axis=AX.X, op=ALU.add)

        ps = pps.tile([1, 2], F32)
        nc.tensor.matmul(ps[:], ones[:], stack2[:], start=True, stop=True)
        fin = cpool.tile([1, 2], F32)
        nc.scalar.copy(fin[:], ps[:])
        nc.sync.dma_start(out_d, fin[:])

    nc.compile()
    return nc


_NC_CACHE = None


def _get_nc():
    global _NC_CACHE
    if _NC_CACHE is None:
        _NC_CACHE = _build_bass()
    return _NC_CACHE


def _rand_modes_full() -> np.ndarray:
    """The reference's fallback modes: jax.random.randint(key(42), (B,), 0, 5)."""
    import jax

    cpu = jax.devices("cpu")[0]
    with jax.default_device(cpu):
        r = jax.random.randint(jax.random.key(42), (B,), 0, NM)
        return np.asarray(jax.device_get(r)).astype(np.float32)


def _make_in_maps(path_pred, path_gt, cr_pred, cr_gt):
    pp = np.ascontiguousarray(np.asarray(path_pred, dtype=np.float32))
    pg = np.ascontiguousarray(
        np.asarray(path_gt, dtype=np.float32).reshape(B, T2)
    )
    crp = np.asarray(cr_pred, dtype=np.float32).reshape(B)
    crg = np.asarray(cr_gt, dtype=np.float32).reshape(B)
    rnd = _rand_modes_full()

    in_maps = []
    for c in range(NCORES):
        sl = slice(c * BLOC, (c + 1) * BLOC)
        rc = (
            rnd[sl]
            .reshape(NSUP, P, G)
            .transpose(1, 0, 2)
            .reshape(P, NJ)
        )
        in_maps.append(
            {
                "path_pred": pp[sl],
                "path_gt": pg[sl],
                "cr_pred": np.ascontiguousarray(crp[sl].reshape(P, NJ)),
                "cr_gt": np.ascontiguousarray(crg[sl].reshape(P, NJ)),
                "rand_modes": np.ascontiguousarray(rc),
            }
        )
    return in_maps


def _combine(results) -> np.float32:
    tot_main = 0.0
    tot_bce = 0.0
    for r in results:
        p = np.asarray(r["partials"], dtype=np.float64)
        tot_main += p[0, 0]
        tot_bce += p[0, 1]
    return np.float32(tot_main / B - tot_bce / B)


def kernel(path_pred, path_gt, cr_pred, cr_gt, log_vars=None, **_ignored):
    in_maps = _make_in_maps(path_pred, path_gt, cr_pred, cr_gt)
    nc = _get_nc()
    res = run_bass_kernel_spmd(nc, in_maps, list(range(NCORES)))
    return _combine(res.results)


def kernel_traced(path_pred, path_gt, cr_pred, cr_gt, log_vars=None, **kw):
    """Like kernel() but with NTFF profiling; returns (loss, BassKernelResults)."""
    in_maps = _make_in_maps(path_pred, path_gt, cr_pred, cr_gt)
    nc = _get_nc()
    res = run_bass_kernel_spmd(nc, in_maps, list(range(NCORES)), trace=True, **kw)
    return _combine(res.results), res
